# revision 32
# baseline (speedup 1.0000x reference)
"""Multi-head attention (B=2, L=2048, D=2048, 16 heads x 128) on 8 trn2 cores.

Sharding: tensor-parallel over heads (4 groups of 4 heads) x data-parallel
over batch (2) -> 8 cores.  Each core computes, for its (batch b, group g):
    hq = q_b @ Wq_g.T, hk = kv_b @ Wk_g.T, hv = kv_b @ Wv_g.T   (4 heads)
    per head: P = softmax(hq hk^T / sqrt(128)), o = P hv
    partial_out = concat_heads(o) @ Wo[:, g].T        [2048, 2048]
Host sums the 4 per-group partials for each batch.

Precision: projections stream bf16 x/w from HBM (halves DMA, keeps the
proj phase under the per-core HBM roofline); hq/hk/hv and the scores/exp/
AV chain stay float32r (TF32-like, full PE rate at free-dim 512); the Wo
path (o, Wo, staged output) is bf16.  Measured end-to-end max-rel ~5e-3.

Per-core schedule (all matmuls bf16, free-dim 512 = 1 PSUM bank, 1 cyc/row,
216ns issue rate):
  projections: 3 passes x q-blocks x 16 contraction chunks x 4 heads.
    Per-chunk weight tiles with DMAs trace-interleaved at pass starts so
    the first matmul waits on ~4 chunks, not the full weight load; x
    superblocks alternate between the sync and scalar DMA queues so each
    queue prefetches one superblock ahead (per-queue sem-counter waits).
    Q-projection q-block 1 is deferred into the attention phase.
  attention per (q-block n, head h), 11 steps:
    steps 0-7: scores pair p (2 matmuls kt=2p,2p+1 -> PSUM pp0) -> exp on
    ACT into bf16 SBUF; steps 3-10: AV pair p-3 (PE accumulates o^T in
    pp1; the 3-pair lag keeps ACT's exp off the PE critical path).
    Softmax denominator: DVE binary-tree sums the bf16 exp tiles (the
    tiles gated on the last exp form the shortest chain); one PE
    ones-matmul folds partitions, deferred into the NEXT iteration's
    step 2 so the PE never waits on the DVE tree; DVE reciprocal +
    scale -> o_sb (bf16).
    n==0: steps {3,5,7,9} run 4 deferred Q-projection matmuls each
    (fills the otherwise ACT-bound first block; hq block 1 is copied
    out on DVE at iteration end, one head per iteration).
    n>0: those slots emit Wo groups for block n-1 (4 matmuls into pp2,
    bf16 stage alternating ACT/DVE, DMA out).
  tail: last block's Wo as 8 x 1024-wide supergroups on the freed pp0.
"""
import math
import sys

for _p in ("/opt/trn_rl_repo", "/root/.axon_site/_ro/trn_rl_repo"):
    if _p not in sys.path:
        sys.path.append(_p)

import numpy as np

B = 2
L = 2048           # LQ == LK
DIN = 2048
NH = 16            # total heads
HL = 4             # heads per core
D = 128            # head dim
HD = HL * D        # 512, head-group width
DOUT = 2048
NC_ = 8            # cores
NCH = DIN // 128   # 16 contraction chunks
NQ = 4             # q blocks of 512
QB = 512
NKT = L // 128     # 16 key tiles

_CACHE = {}


def _build_nc():
    import concourse.bacc as bacc
    import concourse.mybir as mybir
    import concourse.tile as tile

    F32R = mybir.dt.float32r
    F32 = mybir.dt.float32
    BF16 = mybir.dt.bfloat16

    nc = bacc.Bacc("TRN2", target_bir_lowering=False, debug=False)
    qT = nc.dram_tensor("qT", [DIN, L], BF16, kind="ExternalInput").ap()
    kvT = nc.dram_tensor("kvT", [DIN, L], BF16, kind="ExternalInput").ap()
    wqT = nc.dram_tensor("wqT", [DIN, HD], BF16, kind="ExternalInput").ap()
    wkT = nc.dram_tensor("wkT", [DIN, HD], BF16, kind="ExternalInput").ap()
    wvT = nc.dram_tensor("wvT", [DIN, HD], BF16, kind="ExternalInput").ap()
    woT = nc.dram_tensor("woT", [HD, DOUT], BF16, kind="ExternalInput").ap()
    allones = nc.dram_tensor("allones", [128, 128], BF16, kind="ExternalInput").ap()
    out = nc.dram_tensor("out", [L, DOUT], BF16, kind="ExternalOutput").ap()

    EXP = mybir.ActivationFunctionType.Exp
    COPY = mybir.ActivationFunctionType.Copy

    with tile.TileContext(nc) as tc:
        with (
            nc.allow_low_precision(reason="bf16 io + fp32r attention core"),
            tc.tile_pool(name="persist", bufs=1) as pp,
            tc.tile_pool(name="psum", bufs=2, space="PSUM") as psp,
        ):
            hq_sb = pp.tile([128, HL * L], BF16, tag="hq")
            hk_sb = pp.tile([128, HL * L], BF16, tag="hk")
            hv_sb = pp.tile([128, NKT * HD], BF16, tag="hv")
            ones_sb = pp.tile([128, 128], BF16, tag="ones")

            # ---------------- projections ----------------
            # Q-block n=1 is deferred into the attention phase (its matmuls
            # fill the otherwise ACT-bound first attention block), so wq
            # tiles persist and w tags triple-buffer across the 3 passes.
            with tc.tile_pool(name="proj", bufs=1) as jp:
                w_drams = [wqT, wkT, wvT]
                w_tiles = {}

                def w_dma(pass_i, c):
                    t = pp.tile(
                        [128, HD], BF16, tag=f"w{c}", bufs=3, name=f"w{pass_i}_{c}"
                    )
                    nc.gpsimd.dma_start(
                        out=t[:], in_=w_drams[pass_i][c * 128 : (c + 1) * 128, :]
                    )
                    w_tiles[(pass_i, c)] = t

                xq = []
                for pass_i in range(3):
                    x_dram = [qT, kvT, kvT][pass_i]
                    dst = [hq_sb, hk_sb, hv_sb][pass_i]
                    is_v = pass_i == 2
                    if pass_i == 2:
                        # x superblocks for the deferred Q-projection of
                        # q-block 1: stream them during the V pass, on the
                        # otherwise DMA-idle scalar queue so the V pass's own
                        # x stream (sync queue) sees no extra sem counts.
                        for cs in range(4):
                            t = pp.tile(
                                [128, 4 * QB], BF16, tag=f"xq{cs}", bufs=1, name=f"xq{cs}"
                            )
                            nc.gpsimd.dma_start(
                                out=t.rearrange("p (c q) -> p c q", q=QB),
                                in_=qT[cs * 512 : (cs + 1) * 512, QB : 2 * QB].rearrange(
                                    "(c p) q -> p c q", p=128
                                ),
                            )
                            xq.append(t)
                    for n in ((0, 2, 3) if pass_i == 0 else range(NQ)):
                        # j0/j3 share one wide pp0 tile (bank-aligned halves) so
                        # every accumulator tag stays double-buffered across n.
                        acc03 = psp.tile([128, 2 * QB], F32, tag="pp0", name="acc03")
                        acc1 = psp.tile([128, QB], F32, tag="pp1", name="acc1")
                        acc2 = psp.tile([128, QB], F32, tag="pp2", name="acc2")
                        accs = [acc03[:, 0:QB], acc1[:], acc2[:], acc03[:, QB : 2 * QB]]
                        for cs in range(NCH // 4):
                            first = pass_i == 0 and n == 0
                            # cold start: cs1's superblock goes via the gpsimd
                            # queue so it transfers in parallel with cs0's
                            # (back-to-back sync transfers would arrive late).
                            use_g = first and cs == 1
                            if first and not use_g:
                                # first pass: stream weight chunks just ahead of
                                # their matmuls so startup waits ~4 chunks.
                                for ci in range(4):
                                    w_dma(0, cs * 4 + ci)
                            # x superblocks otherwise stay on the sync queue:
                            # ACT/DVE queues stall behind the block-end PSUM
                            # copies (head-of-line), delaying DMA gens.  bufs=6
                            # so a new block's DMA never WARs a buffer freed
                            # only at the end of the previous block.
                            sblk = jp.tile([128, 4 * QB], BF16, tag="blk", bufs=6, name="sblk")
                            (nc.gpsimd if use_g else nc.sync).dma_start(
                                out=sblk.rearrange("p (c q) -> p c q", q=QB),
                                in_=x_dram[
                                    cs * 512 : (cs + 1) * 512, n * QB : (n + 1) * QB
                                ].rearrange("(c p) q -> p c q", p=128),
                            )
                            if use_g:
                                for ci in range(4):
                                    w_dma(0, cs * 4 + ci)
                            for ci in range(4):
                                c = cs * 4 + ci
                                blk = sblk[:, ci * QB : (ci + 1) * QB]
                                w_sb = w_tiles[(pass_i, c)]
                                for j in range(4):
                                    if is_v:
                                        # hv[k, d]: lhsT = kv block cols, rhs = w chunk
                                        nc.tensor.matmul(
                                            accs[j][:],
                                            blk[:, j * 128 : (j + 1) * 128],
                                            w_sb[:],
                                            start=(c == 0),
                                            stop=(c == NCH - 1),
                                        )
                                    else:
                                        # hxT[d, q]: lhsT = w chunk head j, rhs = x block
                                        nc.tensor.matmul(
                                            accs[j][:],
                                            w_sb[:, j * 128 : (j + 1) * 128],
                                            blk[:],
                                            start=(c == 0),
                                            stop=(c == NCH - 1),
                                        )
                        if pass_i < 2 and n == 0:
                            # prefetch next pass's weights during this pass
                            for c in range(NCH):
                                w_dma(pass_i + 1, c)
                            if pass_i == 0:
                                nc.gpsimd.dma_start(out=ones_sb[:], in_=allones)
                        # j0/j3 are the pp0 halves; copy them first, split
                        # over ACT and DVE, so the attention phase's first
                        # pp0 alloc isn't held behind a serial ACT drain.
                        for j in (0, 3, 1, 2):
                            if is_v:
                                # kt = n*4+j holds [128 k, 512(=4h x 128 d)]
                                d_sl = dst[:, (n * 4 + j) * HD : (n * 4 + j + 1) * HD]
                            else:
                                d_sl = dst[:, j * L + n * QB : j * L + (n + 1) * QB]
                            if j in (0, 1):
                                nc.scalar.activation(d_sl, accs[j][:], COPY)
                            else:
                                nc.vector.tensor_copy(out=d_sl, in_=accs[j][:])

            # ---------------- attention + Wo ----------------
            with tc.tile_pool(name="attn", bufs=1) as ap:
                wo_sb = ap.tile([128, HL * DOUT], BF16, tag="wo", bufs=1, name="wo")
                for h in range(HL):
                    nc.gpsimd.dma_start(
                        out=wo_sb[:, h * DOUT : (h + 1) * DOUT],
                        in_=woT[h * 128 : (h + 1) * 128, :],
                    )

                wo_count = [0]

                def emit_wo_group(n_, o_sb_, g):
                    # one Wo output group (qtl, m) for q block n_: 4 matmuls
                    qtl, m = divmod(g, 4)
                    ps_f = psp.tile([128, QB], F32, tag="pp2", name="ps_f")
                    for h_ in range(HL):
                        nc.tensor.matmul(
                            ps_f[:],
                            o_sb_[:, h_ * QB + qtl * 128 : h_ * QB + (qtl + 1) * 128],
                            wo_sb[:, h_ * DOUT + m * QB : h_ * DOUT + (m + 1) * QB],
                            start=(h_ == 0),
                            stop=(h_ == HL - 1),
                        )
                    stage = ap.tile([128, QB], BF16, tag="stage", bufs=3, name="stage")
                    # spread the PSUM->SBUF stage casts evenly over ACT/DVE
                    if wo_count[0] % 2 == 0:
                        nc.scalar.activation(stage[:], ps_f[:], COPY)
                    else:
                        nc.vector.tensor_copy(out=stage[:], in_=ps_f[:])
                    wo_count[0] += 1
                    nc.sync.dma_start(
                        out=out[
                            n_ * QB + qtl * 128 : n_ * QB + (qtl + 1) * 128,
                            m * QB : (m + 1) * QB,
                        ],
                        in_=stage[:],
                    )

                def flush(st, fold_tag="pp0"):
                    # deferred normalization of the previous (n, h) iteration:
                    # fold partitions on PE, reciprocal + scale on DVE.  Runs
                    # mid-next-iteration so the PE never waits on DVE's tree.
                    ps_o_, tr0_, o_sb_, h_ = st
                    fold = psp.tile(
                        [128, 2 * QB] if fold_tag == "pp0" else [128, QB],
                        F32,
                        tag=fold_tag,
                        name="fold",
                    )
                    nc.tensor.matmul(
                        fold[:, 0:QB], ones_sb[:], tr0_[:, 0:QB], start=True, stop=True
                    )
                    recip = ap.tile([128, QB], F32, tag="recip", bufs=2, name="recip")
                    nc.vector.reciprocal_approx_fast(out=recip[:], in_=fold[:, 0:QB])
                    nc.vector.tensor_mul(
                        out=o_sb_[:, h_ * QB : (h_ + 1) * QB],
                        in0=ps_o_[:],
                        in1=recip[:],
                    )

                # Wo slots per head-iteration: (n, 0) slots start late so the
                # previous block's last normalization chain has landed.
                WO_SLOTS = {0: (5, 7, 9), 1: (1, 3, 5, 7, 9), 2: (3, 5, 7, 9), 3: (3, 5, 7, 9)}
                pending = None
                o_tiles = {}
                for n in range(NQ):
                    o_sb = ap.tile([128, HL * QB], BF16, tag="o", bufs=2, name="o")
                    o_tiles[n] = o_sb
                    gi = [0]
                    for h in range(HL):
                        hq_sl = hq_sb[:, h * L + n * QB : h * L + (n + 1) * QB]
                        ps_o = psp.tile([128, QB], F32, tag="pp1", name="ps_o")
                        if n == 0:
                            # deferred Q-projection: head h of q-block 1
                            acc_q = psp.tile([128, QB], F32, tag="pp2", name="acc_q")
                        tr = [
                            ap.tile([128, 4 * QB], BF16, tag=f"tr{i}", bufs=2, name=f"tr{i}")
                            for i in range(2)
                        ]
                        exp_half = [None, None]

                        def e_sl(kt):
                            return exp_half[kt // 8][:, (kt % 8) * QB : (kt % 8 + 1) * QB]

                        def tree(i, lvl):
                            # binary-tree partial sums of exp_half[i] on DVE
                            w = (4 >> lvl) * QB
                            src = exp_half[i] if lvl == 0 else tr[i]
                            nc.vector.tensor_add(
                                out=tr[i][:, 0:w], in0=src[:, 0:w], in1=src[:, w : 2 * w]
                            )

                        # 11 steps: scores/exp for pair p (p<8), AV lagged
                        # three pairs (p>=3) so ACT's exp stays off the PE
                        # critical path.  Softmax denominator: DVE tree-sums
                        # the bf16 exp tiles; the PE partition-fold for the
                        # PREVIOUS iteration is slotted in at p==2.
                        for p in range(11):
                            if p < 8:
                                half = p // 4
                                if p % 4 == 0:
                                    exp_half[half] = ap.tile(
                                        [128, 8 * QB], BF16, tag="exp", bufs=3, name="exp"
                                    )
                                off = (p % 4) * 2 * QB
                                ps_s = psp.tile([128, 2 * QB], F32, tag="pp0", name="ps_s")
                                for t in range(2):
                                    kt = 2 * p + t
                                    nc.tensor.matmul(
                                        ps_s[:, t * QB : (t + 1) * QB],
                                        hk_sb[:, h * L + kt * 128 : h * L + (kt + 1) * 128],
                                        hq_sl,
                                        start=True,
                                        stop=True,
                                    )
                                nc.scalar.activation(
                                    exp_half[half][:, off : off + 2 * QB], ps_s[:], EXP
                                )
                            if p >= 3:
                                for t in range(2):
                                    kt = 2 * (p - 3) + t
                                    nc.tensor.matmul(
                                        ps_o[:],
                                        hv_sb[:, kt * HD + h * 128 : kt * HD + (h + 1) * 128],
                                        e_sl(kt),
                                        start=(kt == 0),
                                        stop=(kt == NKT - 1),
                                    )
                            if p == 2 and pending is not None:
                                flush(pending)
                                pending = None
                            if p in (5, 6, 7):
                                tree(0, p - 5)
                            # second-half tree split so the tiles gated on the
                            # last exp pair form the shortest possible chain
                            if p == 6:
                                # tiles 8-11 (pairs 4,5 exp'd by now)
                                nc.vector.tensor_add(
                                    out=tr[1][:, 0 : 2 * QB],
                                    in0=exp_half[1][:, 0 : 2 * QB],
                                    in1=exp_half[1][:, 2 * QB : 4 * QB],
                                )
                            elif p == 7:
                                nc.vector.tensor_add(
                                    out=tr[1][:, 0:QB],
                                    in0=tr[1][:, 0:QB],
                                    in1=tr[1][:, QB : 2 * QB],
                                )
                            elif p == 8:
                                # tiles 12-15 (pairs 6,7)
                                nc.vector.tensor_add(
                                    out=tr[1][:, 2 * QB : 4 * QB],
                                    in0=exp_half[1][:, 4 * QB : 6 * QB],
                                    in1=exp_half[1][:, 6 * QB : 8 * QB],
                                )
                            elif p == 9:
                                nc.vector.tensor_add(
                                    out=tr[1][:, 2 * QB : 3 * QB],
                                    in0=tr[1][:, 2 * QB : 3 * QB],
                                    in1=tr[1][:, 3 * QB : 4 * QB],
                                )
                                nc.vector.tensor_add(
                                    out=tr[1][:, 0:QB],
                                    in0=tr[1][:, 0:QB],
                                    in1=tr[1][:, 2 * QB : 3 * QB],
                                )
                                nc.vector.tensor_add(
                                    out=tr[0][:, 0:QB],
                                    in0=tr[0][:, 0:QB],
                                    in1=tr[1][:, 0:QB],
                                )
                            if n == 0:
                                if p in (3, 5, 7, 9):
                                    # 4 deferred Q-projection matmuls per slot
                                    si = (p - 3) // 2
                                    for ci in range(4):
                                        c = si * 4 + ci
                                        nc.tensor.matmul(
                                            acc_q[:],
                                            w_tiles[(0, c)][:, h * 128 : (h + 1) * 128],
                                            xq[si][:, ci * QB : (ci + 1) * QB],
                                            start=(c == 0),
                                            stop=(c == NCH - 1),
                                        )
                            elif p in WO_SLOTS[h]:
                                emit_wo_group(n - 1, o_tiles[n - 1], gi[0])
                                gi[0] += 1
                        if n == 0:
                            # hq for q-block 1, head h (read from (1, h) on)
                            nc.vector.tensor_copy(
                                out=hq_sb[:, h * L + QB : h * L + 2 * QB],
                                in_=acc_q[:],
                            )
                        pending = (ps_o, tr[0], o_sb, h)
                    if n > 0:
                        o_tiles.pop(n - 1)
                # tail: the last block's Wo groups run with no scores left, so
                # pair them into 1024-wide supergroups on the freed pp0 banks.
                # The first two supergroups accumulate heads 0-2 BEFORE the
                # final flush so its fold (which waits on the DVE tree) never
                # blocks the in-order PE queue; their head-3 pairs follow the
                # flush.  The flush folds into pp1 since both pp0 bufs are
                # held open across it.
                o_last = o_tiles.pop(NQ - 1)
                n_ = NQ - 1

                def tail_mm(ps_f, sg, h_, stop):
                    qtl, mp = divmod(sg, 2)
                    for t in range(2):
                        m = 2 * mp + t
                        nc.tensor.matmul(
                            ps_f[:, t * QB : (t + 1) * QB],
                            o_last[:, h_ * QB + qtl * 128 : h_ * QB + (qtl + 1) * 128],
                            wo_sb[:, h_ * DOUT + m * QB : h_ * DOUT + (m + 1) * QB],
                            start=(h_ == 0),
                            stop=stop,
                        )

                def tail_out(ps_f, sg):
                    qtl, mp = divmod(sg, 2)
                    stage = ap.tile([128, 2 * QB], BF16, tag="stage2", bufs=2, name="stage2")
                    if sg % 2 == 0:
                        nc.scalar.activation(stage[:], ps_f[:], COPY)
                    else:
                        nc.vector.tensor_copy(out=stage[:], in_=ps_f[:])
                    nc.sync.dma_start(
                        out=out[
                            n_ * QB + qtl * 128 : n_ * QB + (qtl + 1) * 128,
                            mp * 2 * QB : (mp + 1) * 2 * QB,
                        ],
                        in_=stage[:],
                    )

                head_ps = []
                for sg in range(2):
                    ps_f = psp.tile([128, 2 * QB], F32, tag="pp0", name="ps_tail")
                    for h_ in range(3):
                        tail_mm(ps_f, sg, h_, stop=False)
                    head_ps.append(ps_f)
                flush(pending, fold_tag="pp1")
                for sg in range(2):
                    tail_mm(head_ps[sg], sg, 3, stop=True)
                    tail_out(head_ps[sg], sg)
                for sg in range(2, 8):
                    ps_f = psp.tile([128, 2 * QB], F32, tag="pp0", name="ps_tail")
                    for h_ in range(HL):
                        tail_mm(ps_f, sg, h_, stop=(h_ == HL - 1))
                    tail_out(ps_f, sg)
    nc.compile()
    return nc


def _get_nc():
    if "nc" not in _CACHE:
        _CACHE["nc"] = _build_nc()
    return _CACHE["nc"]


def make_in_maps(query, key_value, Wq, Wk, Wv, Wo):
    import ml_dtypes

    bf16 = ml_dtypes.bfloat16
    scale = 1.0 / math.sqrt(D)
    allones = np.ones((128, 128), bf16)
    in_maps = []
    qT = [np.ascontiguousarray(query[b].T.astype(bf16)) for b in range(B)]
    kvT = [np.ascontiguousarray(key_value[b].T.astype(bf16)) for b in range(B)]
    for core in range(NC_):
        b, g = divmod(core, NC_ // B)
        sl = slice(g * HD, (g + 1) * HD)
        in_maps.append(
            {
                "qT": qT[b],
                "kvT": kvT[b],
                "wqT": np.ascontiguousarray((Wq[sl, :] * scale).T.astype(bf16)),
                "wkT": np.ascontiguousarray(Wk[sl, :].T.astype(bf16)),
                "wvT": np.ascontiguousarray(Wv[sl, :].T.astype(bf16)),
                "woT": np.ascontiguousarray(Wo[:, sl].T.astype(bf16)),
                "allones": allones,
            }
        )
    return in_maps


def _numpy_fallback(query, key_value, attention_mask, Wq, Wk, Wv, Wo):
    # Only reached if the mask is not all-ones (never per the problem spec).
    q64, kv64 = query.astype(np.float64), key_value.astype(np.float64)
    hq = (q64 @ Wq.T.astype(np.float64)).reshape(B, L, NH, D).transpose(0, 2, 1, 3)
    hk = (kv64 @ Wk.T.astype(np.float64)).reshape(B, L, NH, D).transpose(0, 2, 1, 3)
    hv = (kv64 @ Wv.T.astype(np.float64)).reshape(B, L, NH, D).transpose(0, 2, 1, 3)
    s = np.einsum("bhqd,bhkd->bhqk", hq, hk) / math.sqrt(D)
    mask = attention_mask[:, None, :, :]
    s = np.where(mask, s, -np.inf)
    s = s - s.max(axis=-1, keepdims=True)
    e = np.exp(s)
    p = e / np.maximum(e.sum(axis=-1, keepdims=True), 1e-300)
    p = np.where(mask, p, 0.0)
    o = np.einsum("bhqk,bhkd->bhqd", p, hv)
    o = o.transpose(0, 2, 1, 3).reshape(B, L, NH * D)
    return (o @ Wo.T.astype(np.float64)).astype(np.float32)


def kernel(query, key_value, attention_mask, Wq, Wk, Wv, Wo):
    query = np.asarray(query)
    key_value = np.asarray(key_value)
    attention_mask = np.asarray(attention_mask)
    Wq, Wk, Wv, Wo = (np.asarray(a) for a in (Wq, Wk, Wv, Wo))

    if not attention_mask.all():
        return _numpy_fallback(query, key_value, attention_mask, Wq, Wk, Wv, Wo)

    from concourse.bass_utils import run_bass_kernel_spmd

    nc = _get_nc()
    in_maps = make_in_maps(query, key_value, Wq, Wk, Wv, Wo)
    res = run_bass_kernel_spmd(nc, in_maps, list(range(NC_))).results
    out = np.zeros((B, L, DOUT), np.float32)
    for core in range(NC_):
        b = core // (NC_ // B)
        out[b] += np.asarray(res[core]["out"]).astype(np.float32)
    return out
